# revision 68
# baseline (speedup 1.0000x reference)
"""Trainium2 Bass kernel for nn_BiLSTM pairwise-scores problem.

Math (reference):
  vec  = concat(word_emb[wi], pos_emb[pi], ext_emb[ei])          [512, 425]
  h    = concat(lstm_cell_f(vec), lstm_cell_b(vec))              [512, 200]
  cat  = [h, vec] for t <= 255 else [vec, h]                     [512, 625]
  f    = cat @ w_mlp_in.T + b_mlp_in                             [512, 400]
  out  = tanh((f[:,None,:] + f[None,:,:]) @ w_mlp_out.T + b_out) [512, 512, 42]

Key factorization: (f_i + f_j) @ W.T + b = g'_i + g'_j with
g' = f @ W.T + b/2, so the O(n^2 * 400 * 42) matmul collapses to a
[512, 42] projection plus a pairwise broadcast-add, implemented on the PE
as a single K=43 matmul per output chunk: lhsT = [g'_i rows; ones row],
rhs = [periodic identity rows; g'_j flattened row].

Sharding: 8 cores = 4 i-blocks (128 rows) x 2 j-halves (256 cols).
Each core runs an identical (SPMD) program on a permuted 384-token slice:
cols 0:128 = its i-block tokens, cols 128:384 = its j-half tokens.

Scheduling notes (from trace analysis):
- The PE mostly runs at the 1.2 GHz mid pstate (it sometimes ramps to
  2.4 after ~3us of dense work, and ACT-paced stalls de-ramp it), so
  the program keeps the PE stream dense with useful work only.
- Input rides two parallel HWDGE queues (sync + scalar) so all four
  gate k-tile pairs land by ~12us; the gpsimd queue's HBM reads are
  ~5x slower and only carry the tiny bias tensor.
- Each gate is a single [100, 384] PSUM accumulation group whose
  matmuls run as two column pieces (j then i) — one full-width
  activation per gate keeps the serial ACT chain to h short.
- The pairwise tanh stream on the ACT engine (~10.4us at 1.2 GHz,
  128 lanes x 1 col/cycle) is the hard floor of the tail; everything
  is ordered to start it as early as possible: jc0 -> flat0 DMA in
  flight while jc1/ic run; per-m mlp_out matmuls interleave with the
  evacuations; el (i-block g') is the last prerequisite.
- rr is split into two tiles so the first pairwise chunks depend only
  on flat0; the chunk straddling the halves is issued last in its
  group. Output is written bf16 (host upcasts) to halve the output
  DMA, with small trailing groups so the last DMA is short.
"""

import os
import sys

import numpy as np

for _p in ("/opt/trn_rl_repo", "/root/.axon_site/_ro/trn_rl_repo"):
    if os.path.isdir(_p) and _p not in sys.path:
        sys.path.insert(0, _p)

import ml_dtypes  # noqa: E402

import concourse.bacc as bacc  # noqa: E402
import concourse.bass as bass  # noqa: E402
import concourse.mybir as mybir  # noqa: E402
from concourse.bass_utils import run_bass_kernel_spmd  # noqa: E402
from concourse.tile import TileContext  # noqa: E402

BF16 = mybir.dt.bfloat16
F32 = mybir.dt.float32
FP8 = mybir.dt.float8e4
AF = mybir.ActivationFunctionType

SEQ = 512
D_VEC = 425  # 100 + 25 + 300
NREL = 42
T = 384  # per-core tokens: 128 (i-block) + 256 (j-half)
NFLAT = 256 * NREL  # 10752 = per-core output row length
N_CHUNK = 512
N_CHUNKS = NFLAT // N_CHUNK  # 21
GRP = 4  # pairwise chunks fused per PSUM group / tanh / DMA
IC_PER = 16 * NREL  # 672: replication period for the identity pattern

# K-dim tiling of the 425-dim feature axis. Near-even tiles (107/106/
# 106/106) rather than 128/128/128/41: a <=64-row tile makes the PE drop
# into half-array row-group mode and the mode switch costs ~150-300ns
# per transition, which dwarfs the saved rows.
KS = [(0, 107), (107, 213), (213, 319), (319, 425)]
# gate order in the stacked [425, 600] gate weight: i_f g_f o_f i_b g_b o_b
GATE_FUNCS = [AF.Sigmoid, AF.Tanh, AF.Sigmoid] * 2

# column groups of the per-core token slice for mlp_in: the two j-halves
# first (so their g' projections + flatten DMAs launch as early as
# possible), the i-block last (its matmuls cover the flatten DMA flight).
# (group, col_a, col_b, j_half_or_None)
J_COLGROUPS = [(1, 128, 256, 0), (1, 256, 384, 1)]
I_COLGROUP = (0, 0, 128, None)

# ---- packed bf16 constant layout: [128, NPK] ----
_SEGS = []  # name -> (rows, col_off, width)


def _seg(name, rows, width):
    off = _SEGS[-1][2] + _SEGS[-1][3] if _SEGS else 0
    _SEGS.append((name, rows, off, width))


# The gates run as fp8-e4m3 DoubleRow matmuls (2 MACs/cycle/lane): the
# 425-feature contraction splits into two halves A = 0:213, B = 213:426
# (feature 425 zero-padded) that ride the same partitions; lhsT/rhs
# carry a (two, free) packed layout. The fp8 gate data lives in its own
# packed tensor pk8; bf16 vt stays for mlp_in.
DR_K = [(0, 107), (107, 213)]  # k-tiles within each 213-feature half
# g68 layout per gate: [two, 112] with the halves padded 100->112 bytes
# (dual-fp8 LDWEIGHTS requires the outer free step to be 16B-aligned)
_off = 0
_S8F = []
for _k, (_a, _b) in enumerate(DR_K):
    _S8F.append((f"vt8{_k}", _b - _a, _off, 2 * 384))
    _off += 2 * 384
    _S8F.append((f"g68{_k}", _b - _a, _off, 6 * 224))
    _off += 6 * 224
SEG8 = {s[0]: s for s in _S8F}
NPK8 = _off

# bf16 packed tensor: vt (mlp rhs), then mlp weights ordered by first
# consumption time.
for _k, (_a, _b) in enumerate(KS):
    _seg(f"vt{_k}", _b - _a, 384)
for _a2 in range(2):
    _seg(f"wh1{_a2}", 100, 400)
for _k, (_a, _b) in enumerate(KS):
    _seg(f"wv1{_k}", _b - _a, 400)
_seg("ic", NREL, IC_PER)
for _a2 in range(2):
    _seg(f"wh0{_a2}", 100, 400)
for _k, (_a, _b) in enumerate(KS):
    _seg(f"wv0{_k}", _b - _a, 400)
_seg("wo", 101, 4 * NREL)
SEG = {s[0]: s for s in _SEGS}
NPK = _SEGS[-1][2] + _SEGS[-1][3]
# bf16 input DMA split points: vt block, g1 weights, ic, g0 weights + wo
PK_CUTS = [SEG["wh10"][2], SEG["ic"][2], SEG["wh00"][2], NPK]


def _build_program():
    nc = bacc.Bacc()

    pk_d = nc.dram_tensor("pk", [128, NPK], BF16, kind="ExternalInput")
    pk8_d = nc.dram_tensor("pk8", [128, NPK8], FP8, kind="ExternalInput")
    bias_d = nc.dram_tensor("bias", [100, 11], F32, kind="ExternalInput")
    out_d = nc.dram_tensor("out", [128, NFLAT], BF16, kind="ExternalOutput")

    with TileContext(nc) as tc:
        with (
            tc.tile_pool(name="const", bufs=1) as cp,
            tc.tile_pool(name="work", bufs=3) as wp,
            tc.tile_pool(name="outp", bufs=3) as op_,
        ):
            # -------- input DMAs first (their triggers must precede the
            # ACT table loads in the scalar stream) --------
            # HBM loads ride the sync + scalar queues (both fast; the
            # gpsimd queue's HBM reads measured ~5x slower). The gate
            # pairs alternate between the two queues so all four land by
            # ~11.5us instead of ~14 — every gate group needs all four
            # k-tiles before its activation can run. bias is tiny and
            # rides gpsimd.
            # The fp8 gate data (420KB) rides sync first so the gates
            # start ASAP; the bf16 stream follows ordered by need (vt for
            # mlp_in, then weights). The scalar queue's 2nd transfer
            # showed multi-us completion straggle, so it carries only
            # late-needed blocks.
            pk8 = cp.tile([128, NPK8], FP8, tag="pk8")
            nc.sync.dma_start(out=pk8, in_=pk8_d[:, :])
            pk = cp.tile([128, NPK], BF16, tag="pk")
            qeng = [nc.sync, nc.scalar, nc.sync, nc.scalar]
            prev = 0
            for eng, cut in zip(qeng, PK_CUTS):
                eng.dma_start(out=pk[:, prev:cut], in_=pk_d[:, prev:cut])
                prev = cut
            bias = cp.tile([100, 11], F32, tag="bias")
            nc.gpsimd.dma_start(out=bias, in_=bias_d[:, :])

            # -------- early on-chip init (no DMA deps) --------
            # lhsT of the pairwise matmul: rows 0:42 = g'_i, row 42 = 1.0.
            # DVE partition base must be 32-aligned, so memset 32:43 and let
            # the later g' write overwrite rows 32:42.
            el = cp.tile([NREL + 1, 128], BF16, tag="el")
            nc.vector.memset(el[32 : NREL + 1, :], 1.0)
            # warmup activations absorb the two ACT table-set loads early
            # (they overlap the input DMA flight)
            warmsrc = cp.tile([1, 8], BF16, tag="warmsrc")
            nc.gpsimd.memset(warmsrc, 0.0)
            warm2 = cp.tile([1, 8], F32, tag="warm2")
            nc.scalar.activation(out=warm2, in_=warmsrc, func=AF.Sigmoid)
            nc.scalar.activation(out=warm2, in_=warmsrc, func=AF.Tanh)

            def seg(name):
                _, rows, off, width = SEG[name]
                return pk[0:rows, off : off + width]

            vt = [seg(f"vt{k}") for k in range(4)]

            wh = [[seg(f"wh{g}{a}") for a in range(2)] for g in range(2)]
            wv = [[seg(f"wv{g}{k}") for k in range(4)] for g in range(2)]
            wo = seg("wo")
            ic = seg("ic")

            # pairwise rhs: rows 0:42 = periodic identity, row 42 = g'_j
            # flat. Two separate tiles (one per j-half) so the first
            # pairwise chunks depend only on flat0, not on flat1 — the
            # Tile dependency tracker is tile-granular for the DMA-written
            # row 42. The identity replication is split into 4 column
            # pieces (a single [42, x] DMA engages only 42 partitions).
            HFLAT = NFLAT // 2  # 5376
            rrs = [
                cp.tile([NREL + 1, HFLAT], BF16, tag="rr0", name="rr0"),
                cp.tile([NREL + 1, HFLAT], BF16, tag="rr1", name="rr1"),
            ]
            REP_PIECES = 4
            reps_per = NFLAT // IC_PER // REP_PIECES  # 4 reps = 2688 cols
            for p in range(REP_PIECES):
                ic_rep = bass.AP(
                    tensor=ic.tensor,
                    offset=ic.offset,
                    ap=[ic.ap[0], [0, reps_per], ic.ap[1]],
                )
                half, off = divmod(p * reps_per * IC_PER, HFLAT)
                nc.gpsimd.dma_start(
                    out=rrs[half][0:NREL, off : off + reps_per * IC_PER],
                    in_=ic_rep,
                )

            with tc.tile_pool(name="psum_pre", bufs=1, space="PSUM") as pp:
                # -------- LSTM gates (both dirs, f-gate skipped) --------
                # Per-direction ordering: i, g (then c = sig(i)*tanh(g)
                # and tanh(c) start immediately), then o, then h.
                hh = [
                    cp.tile([100, T], BF16, tag=f"h{d}", name=f"h{d}")
                    for d in range(2)
                ]

                # Each gate is two fp8 DoubleRow matmuls (one per 107/106
                # k-tile): lhsT [K, (2,100)] / rhs [K, (2,384)] carry the
                # A/B feature halves, the PE computes A_w.T@A_x + B_w.T@
                # B_x at 2 MACs/cycle/lane — 4x fewer PE cycles than the
                # bf16 version and ~2x less gate input data.
                def gact(m):
                    pg = pp.tile([100, T], F32, tag="pg", bufs=3, name=f"pg{m}")
                    for kt, (a, b) in enumerate(DR_K):
                        _, rows, v8o, _ = SEG8[f"vt8{kt}"]
                        _, _, g8o, _ = SEG8[f"g68{kt}"]
                        vs = pk8[0:rows, v8o : v8o + 768]
                        gs = pk8[0:rows, g8o : g8o + 6 * 224]
                        lhsT = bass.AP(
                            tensor=gs.tensor,
                            offset=gs.offset + m * 224,
                            ap=[gs.ap[0], [112, 2], [1, 100]],
                        )
                        rhs = bass.AP(
                            tensor=vs.tensor,
                            offset=vs.offset,
                            ap=[vs.ap[0], [384, 2], [1, 384]],
                        )
                        nc.tensor.matmul(
                            pg,
                            lhsT=lhsT,
                            rhs=rhs,
                            start=(kt == 0),
                            stop=(kt == 1),
                            perf_mode=mybir.MatmulPerfMode.DoubleRow,
                        )
                    a_ = wp.tile([100, T], BF16, tag=f"act{m}", name=f"act{m}")
                    nc.scalar.activation(
                        out=a_,
                        in_=pg,
                        func=GATE_FUNCS[m],
                        bias=bias[0:100, m : m + 1],
                        scale=1.0,
                    )
                    return a_

                def gates_both():
                    for d in range(2):
                        si = gact(3 * d)
                        tg = gact(3 * d + 1)
                        c_ = wp.tile([100, T], BF16, tag=f"c{d}", name=f"c{d}")
                        nc.vector.tensor_mul(c_, si, tg)
                        tc_ = wp.tile([100, T], BF16, tag=f"tc{d}", name=f"tc{d}")
                        nc.scalar.activation(out=tc_, in_=c_, func=AF.Tanh)
                        so = gact(3 * d + 2)
                        nc.vector.tensor_mul(hh[d], so, tc_)

                # -------- mlp_in: fT [400, 384], one column group at a
                # time. Per group the first three m-slices' vec matmuls
                # (no h dependency) are issued before any h matmul, so the
                # PE keeps running while the LSTM ACT chain finishes.
                fm = []
                for m in range(4):
                    rows = 101 if m == 3 else 100
                    f_ = cp.tile([rows, T], BF16, tag=f"f{m}")
                    # fm[3] carries an extra all-ones row 100 so the
                    # natural-layout mlp_out can fold +b_out/2 in as a
                    # rank-1 term (wo row 100 holds b_out/2). Memset base
                    # must be 32-aligned: set 96:101, rows 96:100 are
                    # overwritten by the bias adds below.
                    if m == 3:
                        nc.vector.memset(f_[96:101, :], 1.0)
                    fm.append(f_)

                def colgroup(cgi, g, ca, cb, jh, pool, pf_tag, pf_bufs, po_tag):
                    pfs = [None] * 4
                    # the mlp_out projection for this column group: each
                    # m-slice's matmul is emitted right after that slice's
                    # evacuation instead of waiting for all four.
                    if jh is not None:
                        po = pool.tile([128, NREL], F32, tag=po_tag, name=f"png{jh}")
                    else:
                        po = pool.tile([NREL, 128], F32, tag=po_tag, name="pl")

                    def vec_part(m):
                        pf = pool.tile(
                            [100, cb - ca],
                            F32,
                            tag=pf_tag,
                            bufs=pf_bufs,
                            name=f"pf{cgi}_{m}",
                        )
                        pfs[m] = pf
                        ms = slice(m * 100, (m + 1) * 100)
                        for k in range(4):
                            nc.tensor.matmul(
                                pf,
                                lhsT=wv[g][k][:, ms],
                                rhs=vt[k][:, ca:cb],
                                start=(k == 0),
                                stop=False,
                            )

                    def h_part(m):
                        ms = slice(m * 100, (m + 1) * 100)
                        for a in range(2):
                            nc.tensor.matmul(
                                pfs[m],
                                lhsT=wh[g][a][:, ms],
                                rhs=hh[a][:, ca:cb],
                                start=False,
                                stop=(a == 1),
                            )
                        # the i-block evacuations ride the scalar engine
                        # (idle between the gate ACTs and the pairwise
                        # tanh stream; Identity shares the ACT table set
                        # with Sigmoid/Tanh) so the el path doesn't queue
                        # behind the j-half evacuations on vector
                        if jh is not None:
                            nc.vector.tensor_scalar_add(
                                fm[m][0:100, ca:cb],
                                pfs[m],
                                bias[0:100, 6 + m : 7 + m],
                            )
                        else:
                            nc.scalar.add(
                                fm[m][0:100, ca:cb],
                                pfs[m],
                                bias[0:100, 6 + m : 7 + m],
                            )

                    def out_part(m):
                        if jh is not None:
                            kr = 101 if m == 3 else 100
                            nc.tensor.matmul(
                                po,
                                lhsT=fm[m][0:kr, ca:cb],
                                rhs=wo[0:kr, m * NREL : (m + 1) * NREL],
                                start=(m == 0),
                                stop=(m == 3),
                            )
                        else:
                            nc.tensor.matmul(
                                po,
                                lhsT=wo[0:100, m * NREL : (m + 1) * NREL],
                                rhs=fm[m][0:100, 0:128],
                                start=(m == 0),
                                stop=(m == 3),
                            )

                    vec_part(0)
                    vec_part(1)
                    vec_part(2)
                    h_part(0)
                    out_part(0)
                    vec_part(3)
                    h_part(1)
                    out_part(1)
                    h_part(2)
                    out_part(2)
                    h_part(3)
                    out_part(3)

                    if jh is not None:
                        tj = wp.tile([128, NREL], BF16, tag="tj", name=f"tj{jh}")
                        nc.vector.tensor_copy(tj, po)
                        # both flattens on sync (free once the input
                        # stream is done; scalar is busy with gate ACTs)
                        nc.sync.dma_start(
                            out=rrs[jh][NREL : NREL + 1, :],
                            in_=tj,
                        )
                    else:
                        nc.vector.tensor_scalar_add(
                            el[0:NREL, :], po, bias[0:NREL, 10:11]
                        )

                # jc0 first (its flatten DMA gets ~2.6us of flight), then
                # the i-block so el — the last prerequisite of the first
                # pairwise chunk — completes early. jc1/flat1 are only
                # needed from pairwise chunk 10 on: they run INSIDE the
                # pairwise block (from the pair pool) so the pre-pool
                # closure barrier doesn't delay the first pairwise group.
                gates_both()
                colgroup(
                    0, *J_COLGROUPS[0], pool=pp, pf_tag="pf", pf_bufs=4, po_tag="pq"
                )
                colgroup(
                    1, *J_COLGROUPS[1], pool=pp, pf_tag="pf", pf_bufs=4, po_tag="pq"
                )
                colgroup(
                    2, *I_COLGROUP, pool=pp, pf_tag="pf", pf_bufs=4, po_tag="pq"
                )

            # -------- pairwise: tanh(g'_i + g'_j) --------
            # Small first group lets the (pacing) ACT tanh stream start
            # early; small last groups keep the post-tanh DMA tail short.
            # The tanh output is bf16 so the output DMA is half the bytes
            # of fp32. The chunk that straddles the two rr tiles (and so
            # needs flat1) is issued last within its group.
            grp_plan = (1, 4, 4, 4, 4, 3, 1)
            with tc.tile_pool(name="psum_pair", bufs=2, space="PSUM") as pq:
                c = 0
                for gi, nch in enumerate(grp_plan):
                    ppair = pq.tile([128, GRP * N_CHUNK], F32, tag="ppair")
                    base = c * N_CHUNK
                    qorder = list(range(nch))
                    if c * N_CHUNK < HFLAT < (c + nch) * N_CHUNK:
                        # issue the straddling chunk last (flat1 margin)
                        qorder.sort(key=lambda q: (c + q) * N_CHUNK < HFLAT < (c + q + 1) * N_CHUNK)
                    for q in qorder:
                        cb_ = (c + q) * N_CHUNK
                        pieces = [(cb_, N_CHUNK)]
                        if cb_ < HFLAT < cb_ + N_CHUNK:
                            pieces = [(cb_, HFLAT - cb_), (HFLAT, cb_ + N_CHUNK - HFLAT)]
                        # pieces share one PSUM zero region: start on the
                        # first (zeroes the region), stop on the last
                        off = 0
                        for pi_, (pb, pw_) in enumerate(pieces):
                            nc.tensor.matmul(
                                ppair[:, q * N_CHUNK + off : q * N_CHUNK + off + pw_],
                                lhsT=el,
                                rhs=rrs[pb // HFLAT][:, pb % HFLAT : pb % HFLAT + pw_],
                                start=(pi_ == 0),
                                stop=(pi_ == len(pieces) - 1),
                            )
                            off += pw_
                    ot = op_.tile([128, GRP * N_CHUNK], BF16, tag="ot")
                    nc.scalar.activation(
                        out=ot[:, 0 : nch * N_CHUNK],
                        in_=ppair[:, 0 : nch * N_CHUNK],
                        func=AF.Tanh,
                    )
                    nc.sync.dma_start(
                        out=out_d[:, base : base + nch * N_CHUNK],
                        in_=ot[:, 0 : nch * N_CHUNK],
                    )
                    c += nch

    nc.finalize()
    return nc


def _host_prepare(inputs):
    """Gather embeddings + lay out weights; returns per-core in_maps."""
    bf = ml_dtypes.bfloat16
    wi = np.asarray(inputs["word_idx"]).astype(np.int64)
    pi = np.asarray(inputs["pos_idx"]).astype(np.int64)
    ei = np.asarray(inputs["ext_idx"]).astype(np.int64)
    we = np.asarray(inputs["word_emb"], np.float32)
    pe = np.asarray(inputs["pos_emb"], np.float32)
    xe = np.asarray(inputs["ext_emb"], np.float32)
    vec = np.concatenate([we[wi], pe[pi], xe[ei]], axis=-1)  # [512, 425] f32

    w_ih_f = np.asarray(inputs["w_ih_f"], np.float32)
    w_ih_b = np.asarray(inputs["w_ih_b"], np.float32)
    b_f = np.asarray(inputs["b_f"], np.float32)
    b_b = np.asarray(inputs["b_b"], np.float32)
    w_mlp_in = np.asarray(inputs["w_mlp_in"], np.float32)
    b_mlp_in = np.asarray(inputs["b_mlp_in"], np.float32)
    w_mlp_out = np.asarray(inputs["w_mlp_out"], np.float32)
    b_mlp_out = np.asarray(inputs["b_mlp_out"], np.float32)

    # stacked gate weights [425, 600]: i_f g_f o_f i_b g_b o_b (f unused)
    w6 = np.concatenate(
        [
            w_ih_f[0:100],
            w_ih_f[200:300],
            w_ih_f[300:400],
            w_ih_b[0:100],
            w_ih_b[200:300],
            w_ih_b[300:400],
        ],
        axis=0,
    ).T  # [425, 600]

    bias = np.zeros((100, 11), np.float32)
    for m, sl in enumerate(
        [b_f[0:100], b_f[200:300], b_f[300:400], b_b[0:100], b_b[200:300], b_b[300:400]]
    ):
        bias[:, m] = sl
    bias[:, 6:10] = b_mlp_in.reshape(4, 100).T
    bias[0:NREL, 10] = 0.5 * b_mlp_out

    # row 100: b_out/2 for the natural-layout mlp_out rank-1 bias fold
    # (only the m=3 block's slice is ever read at K=101)
    wo = np.zeros((101, 4 * NREL), np.float32)
    wout_t = w_mlp_out.T  # [400, 42]
    for m in range(4):
        wo[0:100, m * NREL : (m + 1) * NREL] = wout_t[m * 100 : (m + 1) * 100]
        wo[100, m * NREL : (m + 1) * NREL] = 0.5 * b_mlp_out

    # periodic identity block for the pairwise broadcast matmul
    ic = np.zeros((NREL, IC_PER), np.float32)
    cols = np.arange(IC_PER)
    ic[cols % NREL, cols] = 1.0

    def halves(hv):
        if hv:  # cat = [h, vec]
            whx = w_mlp_in[:, 0:200].T  # [200, 400] rows = h features
            wvx = w_mlp_in[:, 200:625].T  # [425, 400] rows = vec features
        else:  # cat = [vec, h]
            whx = w_mlp_in[:, 425:625].T
            wvx = w_mlp_in[:, 0:425].T
        return whx, wvx

    def fill(pk, name, arr):
        _, rows, off, width = SEG[name]
        assert arr.shape == (rows, width), (name, arr.shape, rows, width)
        pk[0:rows, off : off + width] = arr

    in_maps = []
    for core in range(8):
        ib, jh = core // 2, core % 2
        toks = np.concatenate(
            [np.arange(ib * 128, (ib + 1) * 128), np.arange(jh * 256, (jh + 1) * 256)]
        )
        vect = vec[toks].T  # [425, 384]
        g0h, g0v = halves(ib < 2)
        g1h, g1v = halves(jh == 0)

        pk = np.zeros((128, NPK), np.float32)
        for k, (a, b) in enumerate(KS):
            fill(pk, f"vt{k}", vect[a:b])
        for g, (gh, gv) in enumerate([(g0h, g0v), (g1h, g1v)]):
            for a in range(2):
                fill(pk, f"wh{g}{a}", gh[a * 100 : (a + 1) * 100])
            for k, (a, b) in enumerate(KS):
                fill(pk, f"wv{g}{k}", gv[a:b])
        fill(pk, "wo", wo)
        fill(pk, "ic", ic)

        # fp8 DoubleRow gate data: features zero-padded to 426 and split
        # into halves A = 0:213, B = 213:426 sharing partitions.
        vect426 = np.zeros((426, 384), np.float32)
        vect426[0:425] = vect
        w6_426 = np.zeros((426, 600), np.float32)
        w6_426[0:425] = w6
        pk8 = np.zeros((128, NPK8), np.float32)
        for kt, (a, b) in enumerate(DR_K):
            _, rows, v8o, _ = SEG8[f"vt8{kt}"]
            _, _, g8o, _ = SEG8[f"g68{kt}"]
            pk8[0:rows, v8o : v8o + 384] = vect426[a:b]
            pk8[0:rows, v8o + 384 : v8o + 768] = vect426[213 + a : 213 + b]
            # gate weights packed [m, two, 112(pad)]: halves 16B-strided
            for m in range(6):
                g0 = g8o + m * 224
                pk8[0:rows, g0 : g0 + 100] = w6_426[a:b, m * 100 : (m + 1) * 100]
                pk8[0:rows, g0 + 112 : g0 + 212] = w6_426[
                    213 + a : 213 + b, m * 100 : (m + 1) * 100
                ]
        in_maps.append(
            dict(
                pk=pk.astype(bf),
                pk8=pk8.astype(ml_dtypes.float8_e4m3fn),
                bias=bias,
            )
        )
    return in_maps


_CACHED_NC = None


def kernel(**inputs):
    global _CACHED_NC
    in_maps = _host_prepare(inputs)
    if _CACHED_NC is None:
        _CACHED_NC = _build_program()
    res = run_bass_kernel_spmd(_CACHED_NC, in_maps, list(range(8)))
    full = np.empty((SEQ, SEQ, NREL), np.float32)
    for core in range(8):
        ib, jh = core // 2, core % 2
        blk = res.results[core]["out"].astype(np.float32).reshape(128, 256, NREL)
        full[ib * 128 : (ib + 1) * 128, jh * 256 : (jh + 1) * 256, :] = blk
    return full


if __name__ == "__main__":
    rng = np.random.default_rng(0)
    demo = dict(
        word_idx=rng.integers(0, 50000, 512),
        pos_idx=rng.integers(0, 48, 512),
        ext_idx=rng.integers(0, 100000, 512),
        word_emb=rng.standard_normal((50000, 100), np.float32) * 0.05,
        pos_emb=rng.standard_normal((48, 25), np.float32) * 0.05,
        ext_emb=rng.standard_normal((100000, 300), np.float32) * 0.05,
        w_ih_f=rng.standard_normal((400, 425), np.float32) * 0.05,
        b_f=rng.standard_normal(400).astype(np.float32) * 0.05,
        w_ih_b=rng.standard_normal((400, 425), np.float32) * 0.05,
        b_b=rng.standard_normal(400).astype(np.float32) * 0.05,
        w_mlp_in=rng.standard_normal((400, 625), np.float32) * 0.05,
        b_mlp_in=rng.standard_normal(400).astype(np.float32) * 0.05,
        w_mlp_out=rng.standard_normal((42, 400), np.float32) * 0.05,
        b_mlp_out=rng.standard_normal(42).astype(np.float32) * 0.05,
    )
    out = kernel(**demo)
    print("out", out.shape, out.dtype, float(np.abs(out).max()))
